# revision 1
# baseline (speedup 1.0000x reference)
"""Causal single-head attention (B=4, S=4096, D=1024, H=128) on 8 trn2 cores.

Sharding: 2 cores per batch.  Core parity p takes every other 128-row
q-block (global q-block = 2v+p).  KV columns are fed to each core in a
parity-permuted order (adjacent 128-blocks swapped for p=1) so that every
core's q-blocks sit at even *virtual* positions — all 8 cores then run one
identical SPMD program with perfectly balanced causal work:
virtual q-chunk j (512 rows) attends virtual kv-chunks 0..2j+1, the last
two of which carry a data-supplied 0/1 mask.

Per-core dataflow (all matmuls fp32r = full PE rate):
  xT tiles ->  KT[h,kv] / V[kv,h] / QT[h,q] projections (biases folded:
               bq,bk via ACT bias; bv,bo folded into a host-side bias)
  scoresT[kv,q] = KT_blk^T @ QT_chunk   (PSUM)
  exp = ACT Exp(scale*s) PSUM->SBUF; diagonal chunks masked by 0/1 multiply
  outT[h,q]  += V_blk^T @ exp           (PSUM accumulate over kv)
  denom[*,q] += ones^T @ exp            (PSUM accumulate, all rows equal)
  out = (outT * 1/denom)^T @ WoT        -> DMA out
Softmax max-subtraction is skipped: logits are ~N(0,0.17) so exp is safe.
"""

import sys

sys.path.insert(0, "/opt/trn_rl_repo")

import numpy as np

import concourse.bass as bass
import concourse.tile as tile
from concourse import mybir
from concourse.vector_clock import ScopedClock

P = 128
D = 1024
S = 4096
B = 4
H = 128
NCORES = 8
SCALE = 1.0 / float(np.sqrt(H))

F32 = mybir.dt.float32
F32R = mybir.dt.float32r

_patched = [False]


def _patch_tile_drain():
    """The walrus build in this container rejects instructions with more
    than one sync-wait command; spread the Tile kernel-tail drain's
    global-clock waits over single-wait nops."""
    if _patched[0]:
        return
    _patched[0] = True

    def _drain_and_barrier(self, tick_clock, wait_clock):
        nc = self.nc
        probe = nc.sync.nop(nofuse=True)
        wait_clock.add_sem_waits(
            probe.ins, ScopedClock({None: tick_clock.global_clock})
        )
        si = probe.ins.sync_info
        waits = list(si.on_wait) if (si and si.on_wait) else []
        if len(waits) > 1:
            si.on_wait = waits[:1]
            for w in waits[1:]:
                n = nc.sync.nop(nofuse=True)
                nsi = n.ins.sync_info
                if nsi is None:
                    n.ins.sync_info = mybir.SyncInfo(on_wait=[w], on_update=[])
                else:
                    nsi.on_wait = [w]
        nc.sync.drain()
        nc.all_engine_barrier()
        popped = nc._tile_sem_poison_stack.pop()
        assert popped is self._sem_poison
        nc.clear_and_free_semaphores(list(self.sems.allocated().values()))
        nc.all_engine_barrier()

    tile.TileContext._drain_and_barrier = _drain_and_barrier


def _split_excess_waits(nc, max_waits=1):
    """Hoist all but max_waits sync-waits from each instruction onto
    same-engine nops placed immediately before it."""
    for fn in nc.m.functions:
        for bb in fn.blocks:
            new_insts = []
            for inst in bb.instructions:
                si = inst.sync_info
                if si is not None and si.on_wait and len(si.on_wait) > max_waits:
                    waits = list(si.on_wait)
                    for w in waits[:-max_waits]:
                        nop = mybir.InstNoOp(
                            name=nc.get_next_instruction_name(),
                            sync_info=mybir.SyncInfo(on_wait=[w], on_update=[]),
                            bass_nofuse=True,
                            engine=inst.engine,
                        )
                        nc.register_instruction(nop)
                        new_insts.append(nop)
                    si.on_wait = waits[-max_waits:]
                new_insts.append(inst)
            bb.instructions[:] = new_insts


def build_program(d=D, s=S):
    """One uniform per-core program; differences between cores live in data."""
    _patch_tile_drain()
    from contextlib import ExitStack

    DC = d // P            # contraction chunks (8)
    NKVB = s // P          # kv 128-blocks (32)
    NSC = s // 512         # kv 512-chunks (8)
    SQ = s // 2            # queries per core (2048)
    NQC = SQ // 512        # q-chunks (4)

    nc = bass.Bass("TRN2", target_bir_lowering=False, debug=False,
                   num_devices=NCORES)

    xT = nc.declare_dram_parameter("xT", [d, s], F32R, isOutput=False)
    wq = nc.declare_dram_parameter("wq", [d, H], F32R, isOutput=False)
    wk = nc.declare_dram_parameter("wk", [d, H], F32R, isOutput=False)
    wv = nc.declare_dram_parameter("wv", [d, H], F32R, isOutput=False)
    wo = nc.declare_dram_parameter("wo", [H, d], F32R, isOutput=False)
    bqd = nc.declare_dram_parameter("bq", [H, 1], F32, isOutput=False)
    bkd = nc.declare_dram_parameter("bk", [H, 1], F32, isOutput=False)
    maskd = nc.declare_dram_parameter("mask", [8, P, 512], F32R, isOutput=False)
    onesd = nc.declare_dram_parameter("ones", [P, P], F32R, isOutput=False)
    outd = nc.declare_dram_parameter("out", [SQ, d], F32, isOutput=True)

    with tile.TileContext(nc) as tc, ExitStack() as ctx:
        singles = ctx.enter_context(tc.tile_pool(name="singles", bufs=1))
        xt_pool = ctx.enter_context(tc.tile_pool(name="xt", bufs=10))
        exp_pool = ctx.enter_context(tc.tile_pool(name="expp", bufs=3))
        misc = ctx.enter_context(tc.tile_pool(name="misc", bufs=3))
        fin_pool = ctx.enter_context(tc.tile_pool(name="fin", bufs=4))
        ps_a = ctx.enter_context(tc.tile_pool(name="psa", bufs=4, space="PSUM"))
        ps_s = ctx.enter_context(tc.tile_pool(name="pss", bufs=2, space="PSUM"))

        # ---- resident inputs ----
        wq_s = singles.tile([P, DC, H], F32R)
        wk_s = singles.tile([P, DC, H], F32R)
        wv_s = singles.tile([P, DC, H], F32R)
        nc.sync.dma_start(out=wq_s[:], in_=wq.rearrange("(c p) h -> p c h", p=P))
        nc.sync.dma_start(out=wk_s[:], in_=wk.rearrange("(c p) h -> p c h", p=P))
        nc.sync.dma_start(out=wv_s[:], in_=wv.rearrange("(c p) h -> p c h", p=P))
        wo_s = singles.tile([P, d], F32R)
        nc.sync.dma_start(out=wo_s[:], in_=wo[:])
        bq_s = singles.tile([P, 1], F32)
        bk_s = singles.tile([P, 1], F32)
        nc.sync.dma_start(out=bq_s[:], in_=bqd[:])
        nc.sync.dma_start(out=bk_s[:], in_=bkd[:])
        mask_s = singles.tile([P, 8, 512], F32R)
        nc.sync.dma_start(out=mask_s[:], in_=maskd.rearrange("e p c -> p e c"))
        ones_s = singles.tile([P, P], F32R)
        nc.sync.dma_start(out=ones_s[:], in_=onesd[:])

        # ---- projection outputs (resident) ----
        KT = singles.tile([P, s], F32R)        # [h, kv]
        Vn = singles.tile([P, NKVB, P], F32R)  # [kv%128, kvblock, h]
        QT = singles.tile([P, SQ], F32R)       # [h, q]

        # ---- phase 1: projections, one pass over xT ----
        for sc in range(NSC):
            xts = []
            for dc in range(DC):
                xt = xt_pool.tile([P, 512], F32R)
                nc.sync.dma_start(
                    out=xt[:],
                    in_=xT[dc * P:(dc + 1) * P, sc * 512:(sc + 1) * 512],
                )
                xts.append(xt)
            # KT chunk
            kt_ps = ps_a.tile([P, 512], F32, tag="b512")
            for dc in range(DC):
                nc.tensor.matmul(out=kt_ps[:], lhsT=wk_s[:, dc, :],
                                 rhs=xts[dc][:], start=(dc == 0),
                                 stop=(dc == DC - 1))
            nc.scalar.activation(
                out=KT[:, sc * 512:(sc + 1) * 512], in_=kt_ps[:],
                func=mybir.ActivationFunctionType.Identity, bias=bk_s[:],
            )
            # V blocks (natural layout)
            for blk in range(4):
                w_blk = sc * 4 + blk
                v_ps = ps_a.tile([P, 512], F32, tag="b512")
                for dc in range(DC):
                    nc.tensor.matmul(
                        out=v_ps[:, :P],
                        lhsT=xts[dc][:, blk * P:(blk + 1) * P],
                        rhs=wv_s[:, dc, :],
                        start=(dc == 0), stop=(dc == DC - 1),
                    )
                nc.vector.tensor_copy(out=Vn[:, w_blk, :], in_=v_ps[:, :P])
            # Q blocks: virtual-even positions 0 and 2 of this chunk
            for bi, blk in enumerate((0, 2)):
                vq = sc * 2 + bi
                q_ps = ps_a.tile([P, 512], F32, tag="b512")
                for dc in range(DC):
                    nc.tensor.matmul(
                        out=q_ps[:, :P],
                        lhsT=wq_s[:, dc, :],
                        rhs=xts[dc][:, blk * P:(blk + 1) * P],
                        start=(dc == 0), stop=(dc == DC - 1),
                    )
                nc.scalar.activation(
                    out=QT[:, vq * P:(vq + 1) * P], in_=q_ps[:, :P],
                    func=mybir.ActivationFunctionType.Identity, bias=bq_s[:],
                )

        # ---- phase 2: attention + output projection per q-chunk ----
        for j in range(NQC):
            nkv = 2 * j + 2           # kv 512-chunks attended
            npairs = 2 * nkv          # score tiles of 2 kv-blocks each
            nblk = 4 * nkv            # kv 128-blocks attended
            qs = slice(j * 512, (j + 1) * 512)

            acc_ps = ps_a.tile([P, 512], F32, tag="b512")  # outT accumulator
            den_ps = ps_a.tile([P, 512], F32, tag="b512")  # denominator rows

            for m in range(npairs):
                sc_ps = ps_s.tile([P, 2, 512], F32)
                for t in range(2):
                    kvb = 2 * m + t
                    nc.tensor.matmul(
                        out=sc_ps[:, t, :],
                        lhsT=KT[:, kvb * P:(kvb + 1) * P],
                        rhs=QT[:, qs],
                        start=True, stop=True,
                    )
                ex = exp_pool.tile([P, 2, 512], F32R)
                nc.scalar.activation(
                    out=ex[:], in_=sc_ps[:],
                    func=mybir.ActivationFunctionType.Exp, scale=SCALE,
                )
                if m >= npairs - 4:   # last two kv-chunks carry the mask
                    e0 = 2 * (m - (npairs - 4))
                    nc.vector.tensor_mul(
                        out=ex[:], in0=ex[:], in1=mask_s[:, e0:e0 + 2, :]
                    )
                for t in range(2):
                    kvb = 2 * m + t
                    nc.tensor.matmul(
                        out=acc_ps[:], lhsT=Vn[:, kvb, :], rhs=ex[:, t, :],
                        start=(kvb == 0), stop=(kvb == nblk - 1),
                    )
                    nc.tensor.matmul(
                        out=den_ps[:], lhsT=ones_s[:], rhs=ex[:, t, :],
                        start=(kvb == 0), stop=(kvb == nblk - 1),
                    )

            recip = misc.tile([P, 512], F32)
            nc.vector.reciprocal(out=recip[:], in_=den_ps[:])
            otn = misc.tile([P, 512], F32R)
            nc.vector.tensor_mul(out=otn[:], in0=acc_ps[:], in1=recip[:])

            for blk in range(4):
                vq = 4 * j + blk
                fin = fin_pool.tile([P, d], F32)
                for half in range(d // 512):
                    fo_ps = ps_a.tile([P, 512], F32, tag="b512")
                    nc.tensor.matmul(
                        out=fo_ps[:],
                        lhsT=otn[:, blk * P:(blk + 1) * P],
                        rhs=wo_s[:, half * 512:(half + 1) * 512],
                        start=True, stop=True,
                    )
                    nc.any.tensor_copy(
                        out=fin[:, half * 512:(half + 1) * 512], in_=fo_ps[:]
                    )
                nc.sync.dma_start(
                    out=outd[vq * P:(vq + 1) * P, :], in_=fin[:]
                )

    _split_excess_waits(nc)
    return nc


def make_masks(p, dtype=np.float32):
    """mask[e, t, 128u+r] = 1 iff virtual-kv (block e, offset t) is attended
    by virtual-q (block u, offset r) of the same 512-aligned q-chunk."""
    e = np.arange(8)[:, None, None]
    t = np.arange(P)[None, :, None]
    sq = np.arange(512)[None, None, :]
    u, r = sq // P, sq % P
    kv_pos = (e ^ p) * P + t
    q_pos = 256 * u + P * p + r
    return (kv_pos <= q_pos).astype(dtype)


def shard_inputs(x, Wq, bq, Wk, bk, Wv, bv, Wo, bo):
    """Build per-core input maps (and the host-side residual bias)."""
    x = np.asarray(x, dtype=np.float32)
    wq_t = np.ascontiguousarray(np.asarray(Wq, np.float32).T)  # [D, H]
    wk_t = np.ascontiguousarray(np.asarray(Wk, np.float32).T)
    wv_t = np.ascontiguousarray(np.asarray(Wv, np.float32).T)
    wo_t = np.ascontiguousarray(np.asarray(Wo, np.float32).T)  # [H, D]
    bq_c = np.asarray(bq, np.float32).reshape(H, 1)
    bk_c = np.asarray(bk, np.float32).reshape(H, 1)
    ones = np.ones((P, P), np.float32)
    masks = [make_masks(0), make_masks(1)]
    # bv and bo are applied on the host: softmax rows sum to one, so
    # attn@(V+bv) @ Wo^T + bo = attn@V @ Wo^T + (Wo@bv + bo).
    bo_eff = (np.asarray(Wo, np.float32) @ np.asarray(bv, np.float32)
              + np.asarray(bo, np.float32))

    in_maps = []
    nblk = S // P
    for c in range(NCORES):
        b, p = c // 2, c % 2
        xb = x[b]
        if p:
            perm = np.arange(nblk) ^ 1
            xb = xb.reshape(nblk, P, D)[perm].reshape(S, D)
        xT = np.ascontiguousarray(xb.T)
        in_maps.append({
            "xT": xT, "wq": wq_t, "wk": wk_t, "wv": wv_t, "wo": wo_t,
            "bq": bq_c, "bk": bk_c, "mask": masks[p], "ones": ones,
        })
    return in_maps, bo_eff


def gather_outputs(results, bo_eff):
    out = np.empty((B, S, D), np.float32)
    for c in range(NCORES):
        b, p = c // 2, c % 2
        co = results[c]["out"]           # [S//2, D]
        blocks = co.reshape(S // 2 // P, P, D)
        out[b, :, :].reshape(S // P, P, D)[2 * np.arange(S // 2 // P) + p] = blocks
    out += bo_eff[None, None, :]
    return out


_prog_cache = {}


def _get_program():
    if "nc" not in _prog_cache:
        _prog_cache["nc"] = build_program()
    return _prog_cache["nc"]


def kernel(x, Wq, bq, Wk, bk, Wv, bv, Wo, bo):
    from concourse.bass_utils import run_bass_kernel_spmd

    nc = _get_program()
    in_maps, bo_eff = shard_inputs(x, Wq, bq, Wk, bk, Wv, bv, Wo, bo)
    res = run_bass_kernel_spmd(nc, in_maps, core_ids=list(range(NCORES)))
    return gather_outputs(res.results, bo_eff)


# revision 11
# speedup vs baseline: 1.5925x; 1.5925x over previous
"""Causal single-head attention (B=4, S=4096, D=1024, H=128) on 8 trn2 cores.

Sharding: 2 cores per batch.  Core parity p takes every other 128-row
q-block (global q-block = 2v+p).  KV columns are fed to each core in a
parity-permuted order (adjacent 128-blocks swapped for p=1) so that every
core's q-blocks sit at even *virtual* positions — all 8 cores then run one
identical SPMD program with perfectly balanced causal work:
virtual q-chunk j (512 rows) attends virtual kv-chunks 0..2j+1, the last
two of which carry a data-supplied 0/1 mask.

Per-core dataflow (all matmuls fp32r = full PE rate):
  xT tiles ->  KT[h,kv] / V[kv,h] / QT[h,q] projections (biases folded:
               bq,bk via ACT bias; bv,bo folded into a host-side bias)
  scoresT[kv,q] = KT_blk^T @ QT_chunk   (PSUM)
  exp = ACT Exp(scale*s) PSUM->SBUF; diagonal chunks masked by 0/1 multiply
  outT[h,q]  += V_blk^T @ exp           (PSUM accumulate over kv)
  denom[*,q] += ones^T @ exp            (PSUM accumulate, all rows equal)
  out = (outT * 1/denom)^T @ WoT        -> DMA out
Softmax max-subtraction is skipped: logits are ~N(0,0.17) so exp is safe.
"""

import sys

sys.path.insert(0, "/opt/trn_rl_repo")

import numpy as np

import concourse.bass as bass
import concourse.tile as tile
from concourse import mybir
from concourse.vector_clock import ScopedClock

P = 128
D = 1024
S = 4096
B = 4
H = 128
NCORES = 8
SCALE = 1.0 / float(np.sqrt(H))

F32 = mybir.dt.float32
F32R = mybir.dt.float32r

_patched = [False]


def _patch_tile_drain():
    """The walrus build in this container rejects instructions with more
    than one sync-wait command; spread the Tile kernel-tail drain's
    global-clock waits over single-wait nops."""
    if _patched[0]:
        return
    _patched[0] = True

    def _drain_and_barrier(self, tick_clock, wait_clock):
        nc = self.nc
        probe = nc.sync.nop(nofuse=True)
        wait_clock.add_sem_waits(
            probe.ins, ScopedClock({None: tick_clock.global_clock})
        )
        si = probe.ins.sync_info
        waits = list(si.on_wait) if (si and si.on_wait) else []
        if len(waits) > 1:
            si.on_wait = waits[:1]
            for w in waits[1:]:
                n = nc.sync.nop(nofuse=True)
                nsi = n.ins.sync_info
                if nsi is None:
                    n.ins.sync_info = mybir.SyncInfo(on_wait=[w], on_update=[])
                else:
                    nsi.on_wait = [w]
        nc.sync.drain()
        nc.all_engine_barrier()
        popped = nc._tile_sem_poison_stack.pop()
        assert popped is self._sem_poison
        nc.clear_and_free_semaphores(list(self.sems.allocated().values()))
        nc.all_engine_barrier()

    tile.TileContext._drain_and_barrier = _drain_and_barrier


def _split_excess_waits(nc, max_waits=1):
    """Hoist all but max_waits sync-waits from each instruction onto
    same-engine nops placed immediately before it."""
    for fn in nc.m.functions:
        for bb in fn.blocks:
            new_insts = []
            for inst in bb.instructions:
                si = inst.sync_info
                if si is not None and si.on_wait and len(si.on_wait) > max_waits:
                    waits = list(si.on_wait)
                    for w in waits[:-max_waits]:
                        nop = mybir.InstNoOp(
                            name=nc.get_next_instruction_name(),
                            sync_info=mybir.SyncInfo(on_wait=[w], on_update=[]),
                            bass_nofuse=True,
                            engine=inst.engine,
                        )
                        nc.register_instruction(nop)
                        new_insts.append(nop)
                    si.on_wait = waits[-max_waits:]
                new_insts.append(inst)
            bb.instructions[:] = new_insts


def build_program(d=D, s=S):
    """One uniform per-core program; differences between cores live in data."""
    _patch_tile_drain()
    from contextlib import ExitStack

    DC = d // P            # contraction chunks (8)
    NKVB = s // P          # kv 128-blocks (32)
    NSC = s // 512         # kv 512-chunks (8)
    SQ = s // 2            # queries per core (2048)
    NQC = SQ // 512        # q-chunks (4)

    nc = bass.Bass("TRN2", target_bir_lowering=False, debug=False,
                   num_devices=NCORES)

    xT = nc.declare_dram_parameter("xT", [d, s], F32R, isOutput=False)
    wq = nc.declare_dram_parameter("wq", [d, H], F32R, isOutput=False)
    wk = nc.declare_dram_parameter("wk", [d, H], F32R, isOutput=False)
    wv = nc.declare_dram_parameter("wv", [d, H], F32R, isOutput=False)
    wo = nc.declare_dram_parameter("wo", [H, d], F32R, isOutput=False)
    bqd = nc.declare_dram_parameter("bq", [H, 1], F32, isOutput=False)
    bkd = nc.declare_dram_parameter("bk", [H, 1], F32, isOutput=False)
    maskd = nc.declare_dram_parameter("mask", [8, P, 512], F32R, isOutput=False)
    onesd = nc.declare_dram_parameter("ones", [P, P], F32R, isOutput=False)
    outd = nc.declare_dram_parameter("out", [SQ, d], F32, isOutput=True)

    with tile.TileContext(nc) as tc, ExitStack() as ctx:
        singles = ctx.enter_context(tc.tile_pool(name="singles", bufs=1))
        xt_pool = ctx.enter_context(tc.tile_pool(name="xt", bufs=10))
        exp_pool = ctx.enter_context(tc.tile_pool(name="expp", bufs=4))
        misc = ctx.enter_context(tc.tile_pool(name="misc", bufs=3))
        fin_pool = ctx.enter_context(tc.tile_pool(name="fin", bufs=4))
        ps_a = ctx.enter_context(tc.tile_pool(name="psa", bufs=4, space="PSUM"))
        ps_s = ctx.enter_context(tc.tile_pool(name="pss", bufs=2, space="PSUM"))

        # ---- inputs needed by phase 1 (issue these DMAs first) ----
        wk_s = singles.tile([P, DC, H], F32R)
        wq_s = singles.tile([P, DC, H], F32R)
        wv_s = singles.tile([P, DC, H], F32R)
        for dc in range(DC):
            nc.sync.dma_start(out=wk_s[:, dc, :], in_=wk[dc * P:(dc + 1) * P, :])
            nc.scalar.dma_start(out=wv_s[:, dc, :], in_=wv[dc * P:(dc + 1) * P, :])
            nc.scalar.dma_start(out=wq_s[:, dc, :], in_=wq[dc * P:(dc + 1) * P, :])
        bq_s = singles.tile([P, 1], F32)
        bk_s = singles.tile([P, 1], F32)
        nc.sync.dma_start(out=bq_s[:], in_=bqd[:])
        nc.sync.dma_start(out=bk_s[:], in_=bkd[:])

        # ---- projection outputs (resident) ----
        KT = singles.tile([P, s], F32R)        # [h, kv]
        Vn = singles.tile([P, NKVB, P], F32R)  # [kv%128, kvblock, h]
        QT = singles.tile([P, SQ], F32R)       # [h, q]

        # ---- phase 1: projections, one pass over xT ----
        # 1024-wide x tiles halve the DMA instruction count; loads alternate
        # between the two HWDGE issuing engines (SP and ACT) so transfers
        # overlap.
        for s2 in range(NSC // 2):
            xts = []
            for dc in range(DC):
                xt = xt_pool.tile([P, 1024], F32R)
                eng = nc.sync if dc % 2 == 0 else nc.scalar
                eng.dma_start(
                    out=xt[:],
                    in_=xT[dc * P:(dc + 1) * P, s2 * 1024:(s2 + 1) * 1024],
                )
                xts.append(xt)
            for c in range(2):
                sc = 2 * s2 + c
                off = c * 512
                # KT chunk
                kt_ps = ps_a.tile([P, 512], F32, tag="b512")
                for dc in range(DC):
                    nc.tensor.matmul(out=kt_ps[:], lhsT=wk_s[:, dc, :],
                                     rhs=xts[dc][:, off:off + 512],
                                     start=(dc == 0), stop=(dc == DC - 1))
                nc.scalar.activation(
                    out=KT[:, sc * 512:(sc + 1) * 512], in_=kt_ps[:],
                    func=mybir.ActivationFunctionType.Identity, bias=bk_s[:],
                )
                # V blocks (natural layout)
                for blk in range(4):
                    w_blk = sc * 4 + blk
                    v_ps = ps_a.tile([P, 512], F32, tag="b512")
                    for dc in range(DC):
                        nc.tensor.matmul(
                            out=v_ps[:, :P],
                            lhsT=xts[dc][:, off + blk * P:off + (blk + 1) * P],
                            rhs=wv_s[:, dc, :],
                            start=(dc == 0), stop=(dc == DC - 1),
                        )
                    nc.vector.tensor_copy(out=Vn[:, w_blk, :], in_=v_ps[:, :P])
                # Q blocks: virtual-even positions 0 and 2 of this chunk,
                # fused into one N=256 matmul via a strided rhs AP
                vq = sc * 2
                q_ps = ps_a.tile([P, 512], F32, tag="b512")
                for dc in range(DC):
                    rhs4 = xts[dc][:, off:off + 512].rearrange(
                        "p (b c) -> p b c", c=P)
                    nc.tensor.matmul(
                        out=q_ps[:, :2 * P].rearrange("p (b c) -> p b c", c=P),
                        lhsT=wq_s[:, dc, :],
                        rhs=rhs4[:, ::2, :],
                        start=(dc == 0), stop=(dc == DC - 1),
                    )
                nc.scalar.activation(
                    out=QT[:, vq * P:(vq + 2) * P], in_=q_ps[:, :2 * P],
                    func=mybir.ActivationFunctionType.Identity, bias=bq_s[:],
                )

        # ---- inputs only needed by phase 2 (don't block phase-1 DMAs) ----
        wo_s = singles.tile([P, d], F32R)
        nc.sync.dma_start(out=wo_s[:], in_=wo[:])
        mask_s = singles.tile([P, 8, 512], F32R)
        nc.sync.dma_start(out=mask_s[:], in_=maskd.rearrange("e p c -> p e c"))
        ones_s = singles.tile([P, P], F32R)
        nc.sync.dma_start(out=ones_s[:], in_=onesd[:])

        # ---- phase 2: attention + output projection per q-chunk ----
        for j in range(NQC):
            nkv = 2 * j + 2           # kv 512-chunks attended
            npairs = 2 * nkv          # score tiles of 2 kv-blocks each
            nblk = 4 * nkv            # kv 128-blocks attended
            qs = slice(j * 512, (j + 1) * 512)

            acc_ps = ps_a.tile([P, 512], F32, tag="b512")  # outT accumulator
            den_ps = ps_a.tile([P, 512], F32, tag="b512")  # denominator rows

            # Masked (diagonal) blocks first: their extra DVE mask latency then
            # overlaps the remaining unmasked blocks' PE work instead of
            # stalling the tail of the accumulation chain.
            order = list(range(npairs - 4, npairs)) + list(range(npairs - 4))
            for mi, m in enumerate(order):
                sc_ps = ps_s.tile([P, 2, 512], F32)
                for t in range(2):
                    kvb = 2 * m + t
                    nc.tensor.matmul(
                        out=sc_ps[:, t, :],
                        lhsT=KT[:, kvb * P:(kvb + 1) * P],
                        rhs=QT[:, qs],
                        start=True, stop=True,
                    )
                ex = exp_pool.tile([P, 2, 512], F32R)
                nc.scalar.activation(
                    out=ex[:], in_=sc_ps[:],
                    func=mybir.ActivationFunctionType.Exp, scale=SCALE,
                )
                if m >= npairs - 4:   # last two kv-chunks carry the mask
                    e0 = 2 * (m - (npairs - 4))
                    nc.vector.tensor_mul(
                        out=ex[:], in0=ex[:], in1=mask_s[:, e0:e0 + 2, :]
                    )
                for t in range(2):
                    nc.tensor.matmul(
                        out=acc_ps[:], lhsT=Vn[:, 2 * m + t, :], rhs=ex[:, t, :],
                        start=(mi == 0 and t == 0),
                        stop=(mi == npairs - 1 and t == 1),
                    )
                    nc.tensor.matmul(
                        out=den_ps[:], lhsT=ones_s[:], rhs=ex[:, t, :],
                        start=(mi == 0 and t == 0),
                        stop=(mi == npairs - 1 and t == 1),
                    )

            recip = misc.tile([P, 512], F32)
            nc.vector.reciprocal(out=recip[:], in_=den_ps[:])
            otn = misc.tile([P, 512], F32R)
            nc.vector.tensor_mul(out=otn[:], in0=acc_ps[:], in1=recip[:])

            for blk in range(4):
                vq = 4 * j + blk
                fin = fin_pool.tile([P, d], F32)
                for half in range(d // 512):
                    fo_ps = ps_a.tile([P, 512], F32, tag="b512")
                    nc.tensor.matmul(
                        out=fo_ps[:],
                        lhsT=otn[:, blk * P:(blk + 1) * P],
                        rhs=wo_s[:, half * 512:(half + 1) * 512],
                        start=True, stop=True,
                    )
                    nc.vector.tensor_copy(
                        out=fin[:, half * 512:(half + 1) * 512], in_=fo_ps[:]
                    )
                nc.sync.dma_start(
                    out=outd[vq * P:(vq + 1) * P, :], in_=fin[:]
                )

    _split_excess_waits(nc)
    return nc


def make_masks(p, dtype=np.float32):
    """mask[e, t, 128u+r] = 1 iff virtual-kv (block e, offset t) is attended
    by virtual-q (block u, offset r) of the same 512-aligned q-chunk."""
    e = np.arange(8)[:, None, None]
    t = np.arange(P)[None, :, None]
    sq = np.arange(512)[None, None, :]
    u, r = sq // P, sq % P
    kv_pos = (e ^ p) * P + t
    q_pos = 256 * u + P * p + r
    return (kv_pos <= q_pos).astype(dtype)


def shard_inputs(x, Wq, bq, Wk, bk, Wv, bv, Wo, bo):
    """Build per-core input maps (and the host-side residual bias)."""
    x = np.asarray(x, dtype=np.float32)
    wq_t = np.ascontiguousarray(np.asarray(Wq, np.float32).T)  # [D, H]
    wk_t = np.ascontiguousarray(np.asarray(Wk, np.float32).T)
    wv_t = np.ascontiguousarray(np.asarray(Wv, np.float32).T)
    wo_t = np.ascontiguousarray(np.asarray(Wo, np.float32).T)  # [H, D]
    bq_c = np.asarray(bq, np.float32).reshape(H, 1)
    bk_c = np.asarray(bk, np.float32).reshape(H, 1)
    ones = np.ones((P, P), np.float32)
    masks = [make_masks(0), make_masks(1)]
    # bv and bo are applied on the host: softmax rows sum to one, so
    # attn@(V+bv) @ Wo^T + bo = attn@V @ Wo^T + (Wo@bv + bo).
    bo_eff = (np.asarray(Wo, np.float32) @ np.asarray(bv, np.float32)
              + np.asarray(bo, np.float32))

    in_maps = []
    nblk = S // P
    for c in range(NCORES):
        b, p = c // 2, c % 2
        xb = x[b]
        if p:
            perm = np.arange(nblk) ^ 1
            xb = xb.reshape(nblk, P, D)[perm].reshape(S, D)
        xT = np.ascontiguousarray(xb.T)
        in_maps.append({
            "xT": xT, "wq": wq_t, "wk": wk_t, "wv": wv_t, "wo": wo_t,
            "bq": bq_c, "bk": bk_c, "mask": masks[p], "ones": ones,
        })
    return in_maps, bo_eff


def gather_outputs(results, bo_eff):
    out = np.empty((B, S, D), np.float32)
    for c in range(NCORES):
        b, p = c // 2, c % 2
        co = results[c]["out"]           # [S//2, D]
        blocks = co.reshape(S // 2 // P, P, D)
        out[b, :, :].reshape(S // P, P, D)[2 * np.arange(S // 2 // P) + p] = blocks
    out += bo_eff[None, None, :]
    return out


_prog_cache = {}


def _get_program():
    if "nc" not in _prog_cache:
        _prog_cache["nc"] = build_program()
    return _prog_cache["nc"]


def kernel(x, Wq, bq, Wk, bk, Wv, bv, Wo, bo):
    from concourse.bass_utils import run_bass_kernel_spmd

    nc = _get_program()
    in_maps, bo_eff = shard_inputs(x, Wq, bq, Wk, bk, Wv, bv, Wo, bo)
    res = run_bass_kernel_spmd(nc, in_maps, core_ids=list(range(NCORES)))
    return gather_outputs(res.results, bo_eff)


# revision 13
# speedup vs baseline: 2.3664x; 1.4859x over previous
"""Causal single-head attention (B=4, S=4096, D=1024, H=128) on 8 trn2 cores.

Sharding: 2 cores per batch.  Core parity p takes every other 128-row
q-block (global q-block = 2v+p).  KV columns are fed to each core in a
parity-permuted order (adjacent 128-blocks swapped for p=1) so that every
core's q-blocks sit at even *virtual* positions — all 8 cores then run one
identical SPMD program with perfectly balanced causal work:
virtual q-chunk j (512 rows) attends virtual kv-chunks 0..2j+1, the last
two of which carry a data-supplied 0/1 mask.

Per-core dataflow (all matmuls fp32r = full PE rate):
  xT tiles ->  KT[h,kv] / V[kv,h] / QT[h,q] projections (biases folded:
               bq,bk via ACT bias; bv,bo folded into a host-side bias)
  scoresT[kv,q] = KT_blk^T @ QT_chunk   (PSUM)
  exp = ACT Exp(scale*s) PSUM->SBUF; diagonal chunks masked by 0/1 multiply
  outT[h,q]  += V_blk^T @ exp           (PSUM accumulate over kv)
  denom[*,q] += ones^T @ exp            (PSUM accumulate, all rows equal)
  out = (outT * 1/denom)^T @ WoT        -> DMA out
Softmax max-subtraction is skipped: logits are ~N(0,0.17) so exp is safe.
"""

import sys

sys.path.insert(0, "/opt/trn_rl_repo")

import numpy as np

import concourse.bass as bass
import concourse.tile as tile
from concourse import mybir
from concourse.vector_clock import ScopedClock

P = 128
D = 1024
S = 4096
B = 4
H = 128
NCORES = 8
SCALE = 1.0 / float(np.sqrt(H))

F32 = mybir.dt.float32
F32R = mybir.dt.float32r

_patched = [False]


def _patch_tile_drain():
    """The walrus build in this container rejects instructions with more
    than one sync-wait command; spread the Tile kernel-tail drain's
    global-clock waits over single-wait nops."""
    if _patched[0]:
        return
    _patched[0] = True

    def _drain_and_barrier(self, tick_clock, wait_clock):
        nc = self.nc
        probe = nc.sync.nop(nofuse=True)
        wait_clock.add_sem_waits(
            probe.ins, ScopedClock({None: tick_clock.global_clock})
        )
        si = probe.ins.sync_info
        waits = list(si.on_wait) if (si and si.on_wait) else []
        if len(waits) > 1:
            si.on_wait = waits[:1]
            for w in waits[1:]:
                n = nc.sync.nop(nofuse=True)
                nsi = n.ins.sync_info
                if nsi is None:
                    n.ins.sync_info = mybir.SyncInfo(on_wait=[w], on_update=[])
                else:
                    nsi.on_wait = [w]
        nc.sync.drain()
        nc.all_engine_barrier()
        popped = nc._tile_sem_poison_stack.pop()
        assert popped is self._sem_poison
        nc.clear_and_free_semaphores(list(self.sems.allocated().values()))
        nc.all_engine_barrier()

    tile.TileContext._drain_and_barrier = _drain_and_barrier


def _split_excess_waits(nc, max_waits=1):
    """Hoist all but max_waits sync-waits from each instruction onto
    same-engine nops placed immediately before it."""
    for fn in nc.m.functions:
        for bb in fn.blocks:
            new_insts = []
            for inst in bb.instructions:
                si = inst.sync_info
                if si is not None and si.on_wait and len(si.on_wait) > max_waits:
                    waits = list(si.on_wait)
                    for w in waits[:-max_waits]:
                        nop = mybir.InstNoOp(
                            name=nc.get_next_instruction_name(),
                            sync_info=mybir.SyncInfo(on_wait=[w], on_update=[]),
                            bass_nofuse=True,
                            engine=inst.engine,
                        )
                        nc.register_instruction(nop)
                        new_insts.append(nop)
                    si.on_wait = waits[-max_waits:]
                new_insts.append(inst)
            bb.instructions[:] = new_insts


def build_program(d=D, s=S):
    """One uniform per-core program; differences between cores live in data."""
    _patch_tile_drain()
    from contextlib import ExitStack

    DC = d // P            # contraction chunks (8)
    NKVB = s // P          # kv 128-blocks (32)
    NSC = s // 512         # kv 512-chunks (8)
    SQ = s // 2            # queries per core (2048)
    NQC = SQ // 512        # q-chunks (4)

    nc = bass.Bass("TRN2", target_bir_lowering=False, debug=False,
                   num_devices=NCORES)

    xT = nc.declare_dram_parameter("xT", [d, s], F32R, isOutput=False)
    wq = nc.declare_dram_parameter("wq", [d, H], F32R, isOutput=False)
    wk = nc.declare_dram_parameter("wk", [d, H], F32R, isOutput=False)
    wv = nc.declare_dram_parameter("wv", [d, H], F32R, isOutput=False)
    wo = nc.declare_dram_parameter("wo", [H, d], F32R, isOutput=False)
    bqd = nc.declare_dram_parameter("bq", [H, 1], F32, isOutput=False)
    bkd = nc.declare_dram_parameter("bk", [H, 1], F32, isOutput=False)
    maskd = nc.declare_dram_parameter("mask", [8, P, 512], F32R, isOutput=False)
    onesd = nc.declare_dram_parameter("ones", [P, P], F32R, isOutput=False)
    outd = nc.declare_dram_parameter("out", [SQ, d], F32, isOutput=True)

    with tile.TileContext(nc) as tc, ExitStack() as ctx:
        singles = ctx.enter_context(tc.tile_pool(name="singles", bufs=1))
        xt_pool = ctx.enter_context(tc.tile_pool(name="xt", bufs=10))
        exp_pool = ctx.enter_context(tc.tile_pool(name="expp", bufs=4))
        misc = ctx.enter_context(tc.tile_pool(name="misc", bufs=3))
        fin_pool = ctx.enter_context(tc.tile_pool(name="fin", bufs=4))
        ps_a = ctx.enter_context(tc.tile_pool(name="psa", bufs=4, space="PSUM"))
        ps_s = ctx.enter_context(tc.tile_pool(name="pss", bufs=2, space="PSUM"))

        # ---- inputs needed by phase 1 (issue these DMAs first) ----
        wk_s = singles.tile([P, DC, H], F32R)
        wq_s = singles.tile([P, DC, H], F32R)
        wv_s = singles.tile([P, DC, H], F32R)
        for dc in range(DC):
            nc.sync.dma_start(out=wk_s[:, dc, :], in_=wk[dc * P:(dc + 1) * P, :])
            nc.scalar.dma_start(out=wv_s[:, dc, :], in_=wv[dc * P:(dc + 1) * P, :])
            nc.scalar.dma_start(out=wq_s[:, dc, :], in_=wq[dc * P:(dc + 1) * P, :])
        bq_s = singles.tile([P, 1], F32)
        bk_s = singles.tile([P, 1], F32)
        nc.sync.dma_start(out=bq_s[:], in_=bqd[:])
        nc.sync.dma_start(out=bk_s[:], in_=bkd[:])

        # ---- projection outputs (resident) ----
        KT = singles.tile([P, s], F32R)        # [h, kv]
        Vn = singles.tile([P, NKVB, P], F32R)  # [kv%128, kvblock, h]
        QT = singles.tile([P, SQ], F32R)       # [h, q]
        otn_all = singles.tile([P, NQC, 512], F32R)  # normalized outT per j

        # ---- phase 1: projections, one pass over xT ----
        # 1024-wide x tiles halve the DMA instruction count; loads alternate
        # between the two HWDGE issuing engines (SP and ACT) so transfers
        # overlap.
        for s2 in range(NSC // 2):
            xts = []
            for dc in range(DC):
                xt = xt_pool.tile([P, 1024], F32R)
                eng = nc.sync if dc % 2 == 0 else nc.scalar
                eng.dma_start(
                    out=xt[:],
                    in_=xT[dc * P:(dc + 1) * P, s2 * 1024:(s2 + 1) * 1024],
                )
                xts.append(xt)
            for c in range(2):
                sc = 2 * s2 + c
                off = c * 512
                # KT chunk
                kt_ps = ps_a.tile([P, 512], F32, tag="b512")
                for dc in range(DC):
                    nc.tensor.matmul(out=kt_ps[:], lhsT=wk_s[:, dc, :],
                                     rhs=xts[dc][:, off:off + 512],
                                     start=(dc == 0), stop=(dc == DC - 1))
                nc.scalar.activation(
                    out=KT[:, sc * 512:(sc + 1) * 512], in_=kt_ps[:],
                    func=mybir.ActivationFunctionType.Identity, bias=bk_s[:],
                )
                # V blocks (natural layout)
                for blk in range(4):
                    w_blk = sc * 4 + blk
                    v_ps = ps_a.tile([P, 512], F32, tag="b512")
                    for dc in range(DC):
                        nc.tensor.matmul(
                            out=v_ps[:, :P],
                            lhsT=xts[dc][:, off + blk * P:off + (blk + 1) * P],
                            rhs=wv_s[:, dc, :],
                            start=(dc == 0), stop=(dc == DC - 1),
                        )
                    nc.vector.tensor_copy(out=Vn[:, w_blk, :], in_=v_ps[:, :P])
                # Q blocks: virtual-even positions 0 and 2 of this chunk,
                # fused into one N=256 matmul via a strided rhs AP
                vq = sc * 2
                q_ps = ps_a.tile([P, 512], F32, tag="b512")
                for dc in range(DC):
                    rhs4 = xts[dc][:, off:off + 512].rearrange(
                        "p (b c) -> p b c", c=P)
                    nc.tensor.matmul(
                        out=q_ps[:, :2 * P].rearrange("p (b c) -> p b c", c=P),
                        lhsT=wq_s[:, dc, :],
                        rhs=rhs4[:, ::2, :],
                        start=(dc == 0), stop=(dc == DC - 1),
                    )
                nc.scalar.activation(
                    out=QT[:, vq * P:(vq + 2) * P], in_=q_ps[:, :2 * P],
                    func=mybir.ActivationFunctionType.Identity, bias=bq_s[:],
                )

        # ---- inputs only needed by phase 2 (don't block phase-1 DMAs) ----
        wo_s = singles.tile([P, d], F32R)
        nc.sync.dma_start(out=wo_s[:], in_=wo[:])
        mask_s = singles.tile([P, 8, 512], F32R)
        nc.sync.dma_start(out=mask_s[:], in_=maskd.rearrange("e p c -> p e c"))
        ones_s = singles.tile([P, P], F32R)
        nc.sync.dma_start(out=ones_s[:], in_=onesd[:])


        def emit_outproj(jj):
            for blk in range(4):
                vq = 4 * jj + blk
                fin = fin_pool.tile([P, d], F32)
                for half in range(d // 512):
                    fo_ps = ps_a.tile([P, 512], F32, tag="b512")
                    nc.tensor.matmul(
                        out=fo_ps[:],
                        lhsT=otn_all[:, jj, blk * P:(blk + 1) * P],
                        rhs=wo_s[:, half * 512:(half + 1) * 512],
                        start=True, stop=True,
                    )
                    nc.vector.tensor_copy(
                        out=fin[:, half * 512:(half + 1) * 512], in_=fo_ps[:]
                    )
                nc.sync.dma_start(
                    out=outd[vq * P:(vq + 1) * P, :], in_=fin[:]
                )

        # ---- phase 2: attention; outproj(j-1) pipelined into chunk j ----
        for j in range(NQC):
            nkv = 2 * j + 2           # kv 512-chunks attended
            npairs = 2 * nkv          # score tiles of 2 kv-blocks each
            nblk = 4 * nkv            # kv 128-blocks attended
            qs = slice(j * 512, (j + 1) * 512)

            acc_ps = ps_a.tile([P, 512], F32, tag="b512")  # outT accumulator
            den_ps = ps_a.tile([P, 512], F32, tag="b512")  # denominator rows

            # Masked (diagonal) blocks first: their extra DVE mask latency then
            # overlaps the remaining unmasked blocks' PE work instead of
            # stalling the tail of the accumulation chain.
            order = list(range(npairs - 4, npairs)) + list(range(npairs - 4))
            for mi, m in enumerate(order):
                sc_ps = ps_s.tile([P, 2, 512], F32)
                for t in range(2):
                    kvb = 2 * m + t
                    nc.tensor.matmul(
                        out=sc_ps[:, t, :],
                        lhsT=KT[:, kvb * P:(kvb + 1) * P],
                        rhs=QT[:, qs],
                        start=True, stop=True,
                    )
                ex = exp_pool.tile([P, 2, 512], F32R)
                nc.scalar.activation(
                    out=ex[:], in_=sc_ps[:],
                    func=mybir.ActivationFunctionType.Exp, scale=SCALE,
                )
                if m >= npairs - 4:   # last two kv-chunks carry the mask
                    e0 = 2 * (m - (npairs - 4))
                    nc.vector.tensor_mul(
                        out=ex[:], in0=ex[:], in1=mask_s[:, e0:e0 + 2, :]
                    )
                if mi == 2 and j > 0:
                    # previous chunk's output projection: its normalize has
                    # had two pairs of PE work to complete on DVE by now
                    emit_outproj(j - 1)
                for t in range(2):
                    nc.tensor.matmul(
                        out=acc_ps[:], lhsT=Vn[:, 2 * m + t, :], rhs=ex[:, t, :],
                        start=(mi == 0 and t == 0),
                        stop=(mi == npairs - 1 and t == 1),
                    )
                    nc.tensor.matmul(
                        out=den_ps[:], lhsT=ones_s[:], rhs=ex[:, t, :],
                        start=(mi == 0 and t == 0),
                        stop=(mi == npairs - 1 and t == 1),
                    )

            recip = misc.tile([P, 512], F32)
            nc.vector.reciprocal(out=recip[:], in_=den_ps[:])
            nc.vector.tensor_mul(out=otn_all[:, j, :], in0=acc_ps[:],
                                 in1=recip[:])

        # ---- tail: last chunk's output projection ----
        emit_outproj(NQC - 1)

    _split_excess_waits(nc)
    return nc


def make_masks(p, dtype=np.float32):
    """mask[e, t, 128u+r] = 1 iff virtual-kv (block e, offset t) is attended
    by virtual-q (block u, offset r) of the same 512-aligned q-chunk."""
    e = np.arange(8)[:, None, None]
    t = np.arange(P)[None, :, None]
    sq = np.arange(512)[None, None, :]
    u, r = sq // P, sq % P
    kv_pos = (e ^ p) * P + t
    q_pos = 256 * u + P * p + r
    return (kv_pos <= q_pos).astype(dtype)


def shard_inputs(x, Wq, bq, Wk, bk, Wv, bv, Wo, bo):
    """Build per-core input maps (and the host-side residual bias)."""
    x = np.asarray(x, dtype=np.float32)
    wq_t = np.ascontiguousarray(np.asarray(Wq, np.float32).T)  # [D, H]
    wk_t = np.ascontiguousarray(np.asarray(Wk, np.float32).T)
    wv_t = np.ascontiguousarray(np.asarray(Wv, np.float32).T)
    wo_t = np.ascontiguousarray(np.asarray(Wo, np.float32).T)  # [H, D]
    bq_c = np.asarray(bq, np.float32).reshape(H, 1)
    bk_c = np.asarray(bk, np.float32).reshape(H, 1)
    ones = np.ones((P, P), np.float32)
    masks = [make_masks(0), make_masks(1)]
    # bv and bo are applied on the host: softmax rows sum to one, so
    # attn@(V+bv) @ Wo^T + bo = attn@V @ Wo^T + (Wo@bv + bo).
    bo_eff = (np.asarray(Wo, np.float32) @ np.asarray(bv, np.float32)
              + np.asarray(bo, np.float32))

    in_maps = []
    nblk = S // P
    for c in range(NCORES):
        b, p = c // 2, c % 2
        xb = x[b]
        if p:
            perm = np.arange(nblk) ^ 1
            xb = xb.reshape(nblk, P, D)[perm].reshape(S, D)
        xT = np.ascontiguousarray(xb.T)
        in_maps.append({
            "xT": xT, "wq": wq_t, "wk": wk_t, "wv": wv_t, "wo": wo_t,
            "bq": bq_c, "bk": bk_c, "mask": masks[p], "ones": ones,
        })
    return in_maps, bo_eff


def gather_outputs(results, bo_eff):
    out = np.empty((B, S, D), np.float32)
    for c in range(NCORES):
        b, p = c // 2, c % 2
        co = results[c]["out"]           # [S//2, D]
        blocks = co.reshape(S // 2 // P, P, D)
        out[b, :, :].reshape(S // P, P, D)[2 * np.arange(S // 2 // P) + p] = blocks
    out += bo_eff[None, None, :]
    return out


_prog_cache = {}


def _get_program():
    if "nc" not in _prog_cache:
        _prog_cache["nc"] = build_program()
    return _prog_cache["nc"]


def kernel(x, Wq, bq, Wk, bk, Wv, bv, Wo, bo):
    from concourse.bass_utils import run_bass_kernel_spmd

    nc = _get_program()
    in_maps, bo_eff = shard_inputs(x, Wq, bq, Wk, bk, Wv, bv, Wo, bo)
    res = run_bass_kernel_spmd(nc, in_maps, core_ids=list(range(NCORES)))
    return gather_outputs(res.results, bo_eff)
